# revision 25
# baseline (speedup 1.0000x reference)
"""Trainium2 Bass kernel for nn_ClinicalLongformerPool (8 NeuronCores, SPMD).

Sharding: 2-way data parallel over batch x 4-way sequence parallel.
Core c owns tokens [q*1024, (q+1)*1024) of batch b, where b=c//4, q=c%4.
Per layer one 8-core AllGather exchanges 256-token halo edges (+ the 32
global tokens); global-token full-sequence attention is merged flash-style
via a small second AllGather; pooling uses a batch-grouped AllReduce.

Numerics: matmul operands in bf16, accumulation / softmax sums / residual
stream / layernorm in fp32.  Scores are tiny (|s| < ~5) so softmax runs
without a max-subtraction pass; masking adds -30000 before exp (exp -> 0).
"""
import sys
import types

import numpy as np


# ---------------------------------------------------------------- NTFF hook
def _install_ntff_hook():
    try:
        from antenv.axon_hooks import get_axon_ntff_profile_hook  # noqa: F401
        return
    except ImportError:
        pass
    try:
        import antenv
        from trn_agent_boot.trn_boot import _ntff_profile_via_ctypes
    except ImportError:
        return
    mod = types.ModuleType("antenv.axon_hooks")
    _h = [None]
    mod.set_axon_ntff_profile_hook = lambda h: _h.__setitem__(0, h)
    mod.get_axon_ntff_profile_hook = lambda: _h[0]
    sys.modules["antenv.axon_hooks"] = mod
    antenv.axon_hooks = mod
    try:
        mod.set_axon_ntff_profile_hook(
            _ntff_profile_via_ctypes("/opt/axon/libaxon_pjrt.so"))
    except Exception:
        pass


_install_ntff_hook()

import ml_dtypes  # noqa: E402
import concourse.bass as bass  # noqa: E402
import concourse.bacc as bacc  # noqa: E402
import concourse.tile as tile  # noqa: E402
import concourse.mybir as mybir  # noqa: E402
from concourse.bass import ds  # noqa: E402
from concourse.masks import make_identity  # noqa: E402
from concourse.tile_rust import add_dep_helper as _adh  # noqa: E402


def add_dep_helper(a, b, reason=""):
    _adh(getattr(a, "ins", a), getattr(b, "ins", b), reason=reason)

F32 = mybir.dt.float32
BF16 = mybir.dt.bfloat16
I32 = mybir.dt.int32
AF = mybir.ActivationFunctionType
ALU = mybir.AluOpType

# model dims
B, S, D, H, DH, L = 2, 4096, 768, 12, 64, 2
C = 256            # chunk / one-sided window
G = 32             # global tokens
FF = 3072
NL = 25
OWN = S // 4       # 1024 tokens per core
EXT = OWN + 2 * C  # 1536
NT_OWN = OWN // 128   # 8
NT_EXT = EXT // 128   # 12
ND = D // 128         # 6
NF = FF // 128        # 24
NCH_OWN = OWN // C    # 4 chunks per core
NEG = -30000.0
N_CORES = 8

BAND_KTS = (0, 1, 4, 5)   # score k-tiles needing the (static) band mask

DEBUG = False
import os  # noqa: E402
KLEVEL = int(os.environ.get("KLEVEL", "99"))


class _StopEmit(Exception):
    pass


def _stop_if(n):
    if KLEVEL <= n:
        raise _StopEmit()


def _r(w):  # [ (t p), n ] -> [p, t, n] view for K-partition tiling
    return w.rearrange("(t p) n -> p t n", p=128)


def _pbcast(ap, p):
    """Partition-broadcast a [1, ...] AP to [p, ...] (step-0 partition dim)."""
    return bass.AP(tensor=ap.tensor, offset=ap.offset,
                   ap=[[0, p]] + [list(x) for x in ap.ap[1:]])


def _hbcast(ap2):
    """[2, N] AP -> [128, N]: rows 0:64 <- row 0, rows 64:128 <- row 1."""
    s0 = list(ap2.ap[0])[0]
    return bass.AP(tensor=ap2.tensor, offset=ap2.offset,
                   ap=[[s0, 2], [0, 64]] + [list(x) for x in ap2.ap[1:]])


def build_program(trivial, debug=False):
    nc = bacc.Bacc("TRN2", target_bir_lowering=False, debug=False,
                   num_devices=N_CORES)

    def inp(name, shape, dt=F32):
        return nc.dram_tensor(name, list(shape), dt, kind="ExternalInput").ap()

    # replicated weights (host pre-casts the matmul weights to bf16)
    Wq, Wk, Wv = (inp(n, (L, D, D), BF16) for n in ("Wq", "Wk", "Wv"))
    Wqg, Wkg, Wvg = (inp(n, (L, D, D), BF16) for n in ("Wqg", "Wkg", "Wvg"))
    Wo = inp("Wo", (L, D, D), BF16)
    Wf1 = inp("Wf1", (L, D, FF), BF16)
    Wf2 = inp("Wf2", (L, FF, D), BF16)
    bq_s, bqg_s = inp("bq_s", (L, D)), inp("bqg_s", (L, D))  # pre-scaled 1/8
    bk, bkg = inp("bk", (L, D)), inp("bkg", (L, D))
    bv, bvg, bo = inp("bv", (L, D)), inp("bvg", (L, D)), inp("bo", (L, D))
    bf1, bf2 = inp("bf1", (L, FF)), inp("bf2", (L, D))
    ln1_s, ln1_b = inp("ln1_s", (L, D)), inp("ln1_b", (L, D))
    ln2_s, ln2_b = inp("ln2_s", (L, D)), inp("ln2_b", (L, D))
    eln_s, eln_b = inp("emb_ln_s", (D,)), inp("emb_ln_b", (D,))
    clf_w, clf_b = inp("clf_w", (D, NL)), inp("clf_b", (NL,))
    # per-core tensors
    emb_own = inp("emb_own", (OWN, D))
    biascols = inp("biascols", (128, NCH_OWN * 6))   # per (chunk, kt) exp bias
    bandmask = inp("bandmask", (128, 4, C))      # static band masks
    maskG = inp("maskG", (G, 1))                     # global-key mask (additive)
    mask_og = inp("mask_og", (128, NT_OWN))          # og key mask (additive)
    amask = inp("amask", (128, NT_OWN))              # pooling weights (0/1)
    selg_bf = inp("selg_bf", (128, 2), BF16)         # [1-selg, selg]
    selg_f = inp("selg_f", (128, 2))
    clfscale = inp("clfscale", (NL, 1))              # 1/denom
    offs = inp("offs", (1, 12), I32)

    logits_out = nc.dram_tensor("logits_out", [NL, 1], F32,
                                kind="ExternalOutput").ap()
    if debug:
        x_dbg = nc.dram_tensor("x_dbg", [L + 1, EXT, D], F32,
                               kind="ExternalOutput").ap()
        o_dbg = nc.dram_tensor("o_dbg", [L, D, OWN], F32,
                               kind="ExternalOutput").ap()

    groups8 = [list(range(N_CORES))]
    groups_b = [[0, 1, 2, 3], [4, 5, 6, 7]]

    with tile.TileContext(nc) as tc, \
         nc.allow_low_precision(reason="bf16 matmul operands are intentional"), \
         tc.tile_pool(name="persist", bufs=1) as PS, \
         tc.tile_pool(name="acts", bufs=1) as ACTS, \
         tc.tile_pool(name="wpool", bufs=2) as WP, \
         tc.tile_pool(name="tmp", bufs=2) as TMP, \
         tc.tile_pool(name="attp", bufs=6) as ATP, \
         tc.tile_pool(name="ogp", bufs=1) as OGP, \
         tc.tile_pool(name="dram", bufs=1, space="DRAM") as DRAM, \
         tc.tile_pool(name="ps512", bufs=3, space="PSUM") as PSB, \
         tc.tile_pool(name="ps256", bufs=3, space="PSUM") as PSA, \
         tc.tile_pool(name="ps_out", bufs=2, space="PSUM") as PSO:

        # ---------------- persistent small tiles
        ident = PS.tile([128, 128], F32)
        make_identity(nc, ident[:])
        eps_t = PS.tile([128, 1], F32)
        nc.vector.memset(eps_t[:], 1e-5)
        band_sb = PS.tile([128, 4, C], F32)
        nc.sync.dma_start(band_sb[:], bandmask[:])
        bcols_sb = PS.tile([128, NCH_OWN * 6], F32)
        nc.sync.dma_start(bcols_sb[:], biascols[:])
        maskG_sb = PS.tile([G, 1], F32)
        nc.sync.dma_start(maskG_sb[:], maskG[:])
        mog_sb = PS.tile([128, NT_OWN], F32)
        nc.sync.dma_start(mog_sb[:], mask_og[:])
        amask_sb = PS.tile([128, NT_OWN], F32)
        nc.sync.dma_start(amask_sb[:], amask[:])
        selgb_sb = PS.tile([128, 2], BF16)
        nc.sync.dma_start(selgb_sb[:], selg_bf[:])
        selgf_sb = PS.tile([128, 2], F32)
        nc.sync.dma_start(selgf_sb[:], selg_f[:])

        # dynamic offsets
        def load_off(i, maxv):
            regs = nc.alloc_registers(f"off{i}")
            nc.regs_load(regs, offs[0:1, i:i + 1])
            return nc.snap(regs, donate=True, min_val=0, max_val=maxv)

        off_left = load_off(0, N_CORES * 2 * C - C)
        off_right = load_off(1, N_CORES * 2 * C - C)
        off_g32 = load_off(2, N_CORES * 2 * C - G)
        off_fl = [load_off(3 + s, N_CORES * 65 - 65) for s in range(4)]

        # ---------------- big activations
        x_ext = ACTS.tile([128, NT_EXT, D], F32)      # residual stream (+halo)
        xg32 = ACTS.tile([G, D], F32)
        xgT = ACTS.tile([128, ND, G], BF16)
        qgT = ACTS.tile([128, ND, G], BF16)
        kGT = ACTS.tile([128, ND, G], BF16)
        vG_aug = ACTS.tile([G, H, DH + 1], BF16)
        attn_oT = ACTS.tile([128, ND, OWN], BF16)
        sums_sb = ACTS.tile([H, OWN], F32)
        recips = ACTS.tile([H, OWN], BF16)
        og_resh = ACTS.tile([128, ND, G], BF16)

        # aliased groups (disjoint lifetimes share one buffer)
        BIGA = ACTS.tile([128, 12384], BF16)
        kgT = BIGA[:, 0:6144].rearrange("p (k n) -> p k n", k=ND)
        qT = BIGA[:, 0:6144].rearrange("p (k n) -> p k n", k=ND)
        vg_aug = BIGA[:, 6144:12384].rearrange(
            "p (t h d) -> p t h d", t=NT_OWN, h=H)
        recip_bc = BIGA[:, 6144:12288].rearrange("p (k n) -> p k n", k=ND)
        x2 = BIGA[:, 0:12288].bitcast(F32).rearrange(
            "p (t d) -> p t d", t=NT_OWN)          # fp32 view, 8x768

        BIGB = ACTS.tile([128, 15360], BF16)
        xT = BIGB[:, 0:9216].rearrange("p (k n) -> p k n", k=ND)
        attn_stage = BIGB[:, 9216:15360]           # unused scratch
        x2T = BIGB[:, 0:3072].rearrange("p (k n) -> p k n", k=ND)
        h1T = BIGB[:, 3072:15360].rearrange("p (k n) -> p k n", k=NF)

        BIGC = ACTS.tile([128, 18576], BF16)
        kT = BIGC[:, 0:9216].rearrange("p (k n) -> p k n", k=ND)
        v_aug = BIGC[:, 9216:18576].rearrange(
            "p (t h d) -> p t h d", t=NT_EXT, h=H)
        wf2_buf = [BIGC[:, 0:6144].rearrange("p (k n) -> p k n", k=NF),
                   BIGC[:, 6144:12288].rearrange("p (k n) -> p k n", k=NF)]

        # DRAM bounce buffers
        edge_in = DRAM.tile([2 * C, D], F32)
        edge_outs = [DRAM.tile([N_CORES * 2 * C, D], F32, addr_space="Shared",
                               name=f"edge_out{i}") for i in range(L)]
        fl_in = DRAM.tile([65, H * G], F32)
        fl_outs = [DRAM.tile([N_CORES * 65, H * G], F32, addr_space="Shared",
                             name=f"fl_out{i}") for i in range(L)]
        pool_in = DRAM.tile([128, ND], F32)
        pool_out = DRAM.tile([128, ND], F32)
        ogrec_dr = DRAM.tile([1, H * G], F32)
        recips_dr = DRAM.tile([H, OWN], BF16)
        sums_dr = DRAM.tile([H, OWN], F32)

        # ---------------- helpers
        def layernorm(src_ap, dst_ap, s_bc, b_bc):
            stats = TMP.tile([128, 3, 6], F32, tag="ln_stats")
            mv = TMP.tile([128, 2], F32, tag="ln_mv")
            for sg in range(3):
                nc.vector.bn_stats(stats[:, sg, :],
                                   src_ap[:, sg * 256:(sg + 1) * 256])
            nc.vector.bn_aggr(mv[:], stats[:])
            nc.scalar.activation(mv[:, 1:2], mv[:, 1:2], AF.Sqrt,
                                 bias=eps_t[:], scale=1.0)
            nc.vector.reciprocal(mv[:, 1:2], mv[:, 1:2])
            nc.vector.tensor_scalar(dst_ap, src_ap, mv[:, 0:1], mv[:, 1:2],
                                    ALU.subtract, ALU.mult)
            if s_bc is not None:
                nc.vector.tensor_mul(dst_ap, dst_ap, s_bc[:])
            if b_bc is not None:
                nc.vector.tensor_add(dst_ap, dst_ap, b_bc[:])

        def ln_params(s_in, b_in, s_triv, b_triv):
            s_bc = b_bc = None
            if not s_triv:
                s_bc = TMP.tile([128, D], F32, tag="ln_sbc")
                nc.sync.dma_start(s_bc[:], _pbcast(s_in[None, :], 128))
            if not b_triv:
                b_bc = TMP.tile([128, D], F32, tag="ln_bbc")
                nc.sync.dma_start(b_bc[:], _pbcast(b_in[None, :], 128))
            return s_bc, b_bc

        def bias_cols(b_in, triv, nd=ND, tag="bias_cols"):
            if triv:
                return None
            t = TMP.tile([128, nd], F32, tag=tag)
            nc.sync.dma_start(t[:], b_in.rearrange("(t p) -> p t", p=128))
            return t

        def free_bias(b_in, triv, blocks=2):
            if triv:
                return None
            t = TMP.tile([128, blocks, D // blocks], F32,
                         tag=f"free_bias{blocks}")
            nc.sync.dma_start(
                t[:], _pbcast(b_in.rearrange("(a n) -> a n", a=blocks)[None],
                              128))
            return t

        def proj_T(out_t, w, b_col, rhs_t, ntok, scale=1.0, rhs_off=0):
            """out_t[:, m, :ntok] = w[ktile].T @ rhs (+bias, *scale)."""
            wr = _r(w)
            for m in range(ND):
                wt = WP.tile([128, ND, 128], BF16, tag="w_lhsT")
                nc.sync.dma_start(wt[:], wr[:, :, m * 128:(m + 1) * 128])
                for nb0 in range(0, ntok, 512):
                    nn = min(512, ntok - nb0)
                    ps = PSB.tile([128, 512], F32, tag="ps512", name="ps512")
                    for k in range(ND):
                        nc.tensor.matmul(
                            ps[:, :nn], wt[:, k, :],
                            rhs_t[:, k, rhs_off + nb0:rhs_off + nb0 + nn],
                            start=(k == 0), stop=(k == ND - 1))
                    nc.scalar.activation(
                        out_t[:, m, nb0:nb0 + nn], ps[:, :nn], AF.Identity,
                        bias=b_col[:, m:m + 1] if b_col is not None else 0.0,
                        scale=scale)

        def proj_V(out_t, w, lhs_t, ntt, badd, lhs_off=0):
            """v-style: out_t[:, tt, h, 0:64] = x @ w (+b); ones in col 64."""
            wr = _r(w)
            for nb in range(2):
                wt = WP.tile([128, ND, 384], BF16, tag="w_rhs")
                nc.sync.dma_start(wt[:], wr[:, :, nb * 384:(nb + 1) * 384])
                for tt in range(ntt):
                    c0 = lhs_off + tt * 128
                    ps = PSB.tile([128, 512], F32, tag="ps512", name="ps512")[:, 0:384]
                    for k in range(ND):
                        nc.tensor.matmul(
                            ps[:], lhs_t[:, k, c0:c0 + 128],
                            wt[:, k, :], start=(k == 0), stop=(k == ND - 1))
                    if badd is not None:
                        nc.vector.tensor_add(ps[:], ps[:], badd[:, nb, :])
                    nc.vector.tensor_copy(
                        out_t[:, tt, nb * 6:(nb + 1) * 6, 0:DH],
                        ps[:].rearrange("p (h d) -> p h d", d=DH))
            nc.vector.memset(out_t[:, :, :, DH:DH + 1], 1.0)

        def transpose_to(dst, src_fn, ntt, dst_off=0):
            for tt in range(ntt):
                for dt in range(ND):
                    pt = PSA.tile([128, 256], F32, tag="ps256", name="ps256")[:, 0:128]
                    nc.tensor.transpose(pt, src_fn(tt, dt), ident[:])
                    nc.scalar.copy(
                        dst[:, dt, dst_off + tt * 128:dst_off + (tt + 1) * 128],
                        pt)

        # ================= embedding layernorm =================
        es_bc, eb_bc = ln_params(eln_s, eln_b, trivial["emb_ln_s"],
                                 trivial["emb_ln_b"])
        er = emb_own.rearrange("(t p) d -> p t d", p=128)
        for tt in range(NT_OWN):
            et = TMP.tile([128, D], F32, tag="emb_in")
            nc.sync.dma_start(et[:], er[:, tt, :])
            layernorm(et[:], x_ext[:, 2 + tt, :], es_bc, eb_bc)

        # ================= layers =================
        try:
         for l in range(L):
            if l == 1:
                _stop_if(98)
            edge_out, fl_out = edge_outs[l], fl_outs[l]
            # ---- edge exchange (own first/last 256 post-LN rows)
            edge_in_v = edge_in.rearrange("(t p) d -> p t d", p=128)
            nc.sync.dma_start(edge_in_v[:, 0:2, :], x_ext[:, 2:4, :])
            nc.sync.dma_start(edge_in_v[:, 2:4, :], x_ext[:, 8:10, :])
            cc_edge = nc.gpsimd.collective_compute(
                "AllGather", ALU.bypass, replica_groups=groups8,
                ins=[edge_in.opt()], outs=[edge_out.opt()])
            r1 = nc.sync.dma_start(
                x_ext[:, 0:2, :],
                edge_out[ds(off_left, C), :].rearrange("(t p) d -> p t d", p=128))
            r2 = nc.sync.dma_start(
                x_ext[:, 10:12, :],
                edge_out[ds(off_right, C), :].rearrange("(t p) d -> p t d", p=128))
            r3 = nc.sync.dma_start(xg32[:], edge_out[ds(off_g32, G), :])
            for r in (r1, r2, r3):
                add_dep_helper(r, cc_edge, reason="dyn read after AG")

            _stop_if(1)
            if debug:
                nc.sync.dma_start(
                    x_dbg[l].rearrange("(t p) d -> p t d", p=128), x_ext[:])

            # ---- transposes
            transpose_to(xT, lambda tt, dt: x_ext[:, tt, dt * 128:(dt + 1) * 128],
                         NT_EXT)
            for dt in range(ND):
                pt = PSA.tile([128, 256], F32, tag="ps256", name="ps256")[:, 0:G]
                nc.tensor.matmul(pt, xg32[:, dt * 128:(dt + 1) * 128],
                                 ident[0:G, 0:G], is_transpose=True,
                                 start=True, stop=True)
                nc.scalar.copy(xgT[:, dt, :], pt)

            _stop_if(2)
            # ---- og projections first (kgT/vg_aug alias qT/recip_bc)
            proj_T(kgT, Wkg[l], bias_cols(bkg[l], trivial["bkg"]),
                   xT, OWN, rhs_off=C)
            proj_T(qgT, Wqg[l], bias_cols(bqg_s[l], trivial["bqg"]),
                   xgT, G, scale=0.125)
            proj_V(vg_aug, Wvg[l], xT, NT_OWN,
                   free_bias(bvg[l], trivial["bvg"]), lhs_off=C)

            _stop_if(3)
            # ---- og flash partials + AllGather + merge
            og_ps = PSO.tile([65, H, G], F32, tag="ps_out", name="ps_out")
            for h in range(H):
                p0, dt = (h % 2) * 64, h // 2
                for kt in range(NT_OWN):
                    ps_s = PSA.tile([128, 256], F32, tag="ps256", name="ps256")[:, 0:G]
                    nc.tensor.matmul(
                        ps_s, kgT[p0:p0 + 64, dt, kt * 128:(kt + 1) * 128],
                        qgT[p0:p0 + 64, dt, :], start=True, stop=True)
                    pg = ATP.tile([128, G], BF16, tag="og_p")
                    nc.scalar.activation(pg[:], ps_s, AF.Exp,
                                         bias=mog_sb[:, kt:kt + 1], scale=1.0)
                    nc.tensor.matmul(og_ps[:, h, :], vg_aug[:, kt, h, :],
                                     pg[:], start=(kt == 0),
                                     stop=(kt == NT_OWN - 1))
            og_part = OGP.tile([65, H * G], F32, tag="og_part")
            nc.vector.tensor_copy(og_part[:],
                                  og_ps[:].rearrange("p h g -> p (h g)"))
            nc.sync.dma_start(fl_in[:], og_part[:])
            cc_fl = nc.gpsimd.collective_compute(
                "AllGather", ALU.bypass, replica_groups=groups8,
                ins=[fl_in.opt()], outs=[fl_out.opt()])
            og_mrg = OGP.tile([65, H * G], F32, tag="og_mrg")
            for s in range(4):
                og_gath = ATP.tile([65, H * G], F32, tag="og_gath",
                                   name="og_gath")
                rg = nc.sync.dma_start(og_gath[:], fl_out[ds(off_fl[s], 65), :])
                add_dep_helper(rg, cc_fl, reason="dyn read after flash AG")
                if s == 0:
                    nc.vector.tensor_copy(og_mrg[:], og_gath[:])
                else:
                    nc.vector.tensor_add(og_mrg[:], og_mrg[:], og_gath[:])
            og_rec = OGP.tile([1, H * G], F32, tag="og_rec")
            nc.vector.reciprocal(og_rec[:], og_mrg[64:65, :])
            w_ogr = nc.sync.dma_start(ogrec_dr[:], og_rec[:])
            og_rbc = OGP.tile([64, H * G], F32, tag="og_rbc")
            r_ogr = nc.sync.dma_start(og_rbc[:], _pbcast(ogrec_dr[0:1, :], 64))
            add_dep_helper(r_ogr, w_ogr, reason="ogrec bounce")
            og_nrm = OGP.tile([64, H * G], F32, tag="og_nrm")
            nc.vector.tensor_mul(og_nrm[:], og_mrg[0:64, :], og_rbc[:])
            nc.vector.tensor_scalar_mul(og_nrm[:], og_nrm[:],
                                        selgf_sb[0:64, 1:2])
            og_fin = OGP.tile([64, H, G], BF16, tag="og_fin")
            nc.vector.tensor_copy(
                og_fin[:].rearrange("p h g -> p (h g)"), og_nrm[:])
            for h in range(H):
                p0, dt = (h % 2) * 64, h // 2
                nc.sync.dma_start(og_resh[p0:p0 + 64, dt, :], og_fin[:, h, :])

            _stop_if(4)
            # ---- main projections
            proj_T(qT, Wq[l], bias_cols(bq_s[l], trivial["bq"]),
                   xT, OWN, scale=0.125, rhs_off=C)
            proj_T(kT, Wk[l], bias_cols(bk[l], trivial["bk"]), xT, EXT)
            proj_T(kGT, Wk[l], bias_cols(bk[l], trivial["bk"]), xgT, G)
            proj_V(v_aug, Wv[l], xT, NT_EXT, free_bias(bv[l], trivial["bv"]))
            # global v (standard Wv of the 32 global tokens)
            wrv = _r(Wv[l])
            vbias = free_bias(bv[l], trivial["bv"])
            for nb in range(2):
                wt = WP.tile([128, ND, 384], BF16, tag="w_rhs")
                nc.sync.dma_start(wt[:], wrv[:, :, nb * 384:(nb + 1) * 384])
                ps = PSB.tile([128, 512], F32, tag="ps512", name="ps512")[0:G, 0:384]
                for k in range(ND):
                    nc.tensor.matmul(ps, xgT[:, k, :], wt[:, k, :],
                                     start=(k == 0), stop=(k == ND - 1))
                if vbias is not None:
                    nc.vector.tensor_add(ps, ps, vbias[0:G, nb, :])
                nc.vector.tensor_copy(
                    vG_aug[:, nb * 6:(nb + 1) * 6, 0:DH],
                    ps.rearrange("p (h d) -> p h d", d=DH))
            nc.vector.memset(vG_aug[:, :, DH:DH + 1], 1.0)

            _stop_if(5)
            # ---- banded local + global-key attention
            for i in range(NCH_OWN):
                for h in range(H):
                    p0, dt = (h % 2) * 64, h // 2
                    qh = qT[p0:p0 + 64, dt, i * C:(i + 1) * C]
                    ps_o = PSO.tile([65, H, G], F32, tag="ps_out", name="ps_out")
                    ps_o = ps_o[:].rearrange("p h g -> p (h g)")[:, 0:C]
                    for kt in range(6):
                        et = i * 2 + kt
                        ps_s = PSA.tile([128, 256], F32, tag="ps256", name="ps256")
                        nc.tensor.matmul(
                            ps_s[:], kT[p0:p0 + 64, dt, et * 128:(et + 1) * 128],
                            qh, start=True, stop=True)
                        if kt in BAND_KTS:
                            j = BAND_KTS.index(kt)
                            nc.vector.tensor_tensor(
                                ps_s[:], ps_s[:], band_sb[:, j, :], ALU.add)
                        pT = ATP.tile([128, C], BF16, tag="att_p")
                        nc.scalar.activation(
                            pT[:], ps_s[:], AF.Exp,
                            bias=bcols_sb[:, i * 6 + kt:i * 6 + kt + 1],
                            scale=1.0)
                        nc.tensor.matmul(ps_o, v_aug[:, et, h, :], pT[:],
                                         start=(kt == 0), stop=False)
                    ps_g = PSA.tile([128, 256], F32, tag="ps256", name="ps256")[0:G, :]
                    nc.tensor.matmul(ps_g, kGT[p0:p0 + 64, dt, :], qh,
                                     start=True, stop=True)
                    pgT = ATP.tile([G, C], BF16, tag="att_gp")
                    nc.scalar.activation(pgT[:], ps_g, AF.Exp,
                                         bias=maskG_sb[:], scale=1.0)
                    nc.tensor.matmul(ps_o, vG_aug[:, h, :], pgT[:],
                                     start=False, stop=True)
                    nc.scalar.copy(attn_oT[p0:p0 + 64, dt, i * C:(i + 1) * C],
                                   ps_o[0:64, :])
                    sums_tmp = ATP.tile([1, C], F32, tag="sums_tmp",
                                        name="sums_tmp")
                    nc.vector.tensor_copy(sums_tmp[:], ps_o[64:65, :])
                    nc.sync.dma_start(sums_dr[h:h + 1, i * C:(i + 1) * C],
                                      sums_tmp[:])

            # ---- normalize + og blend
            nc.sync.dma_start(sums_sb[:], sums_dr[:])
            nc.vector.reciprocal(recips[:], sums_sb[:])
            w_rec = nc.sync.dma_start(recips_dr[:], recips[:])
            for dt in range(ND):
                r_rec = nc.sync.dma_start(
                    recip_bc[:, dt, :],
                    _hbcast(recips_dr[2 * dt:2 * dt + 2, :]))
                add_dep_helper(r_rec, w_rec, reason="recips bounce")
            nc.vector.tensor_mul(attn_oT[:], attn_oT[:], recip_bc[:])
            nc.vector.tensor_scalar_mul(attn_oT[:, :, 0:G],
                                        attn_oT[:, :, 0:G], selgf_sb[:, 0:1])
            nc.vector.tensor_add(attn_oT[:, :, 0:G], attn_oT[:, :, 0:G],
                                 og_resh[:])

            if debug:
                nc.gpsimd.dma_start(
                    o_dbg[l].rearrange("(k p) n -> p k n", p=128), attn_oT[:])

            _stop_if(6)
            # ---- out-projection + residual + LN1 -> x2 (fp32, aliases BIGA)
            l1s, l1b = ln_params(ln1_s[l], ln1_b[l], trivial["ln1_s"],
                                 trivial["ln1_b"])
            ob = free_bias(bo[l], trivial["bo"])
            wro = _r(Wo[l])
            for nb in range(2):
                wt = WP.tile([128, ND, 384], BF16, tag="w_rhs")
                nc.sync.dma_start(wt[:], wro[:, :, nb * 384:(nb + 1) * 384])
                for tt in range(NT_OWN):
                    ps = PSB.tile([128, 512], F32, tag="ps512", name="ps512")[:, 0:384]
                    for k in range(ND):
                        nc.tensor.matmul(
                            ps, attn_oT[:, k, tt * 128:(tt + 1) * 128],
                            wt[:, k, :], start=(k == 0), stop=(k == ND - 1))
                    if ob is not None:
                        nc.vector.tensor_add(ps, ps, ob[:, nb, :])
                    nc.vector.tensor_add(
                        x2[:, tt, nb * 384:(nb + 1) * 384], ps,
                        x_ext[:, 2 + tt, nb * 384:(nb + 1) * 384])
            for tt in range(NT_OWN):
                layernorm(x2[:, tt, :], x2[:, tt, :], l1s, l1b)

            _stop_if(7)
            # ---- FFN (two token halves) + residual + LN2 -> x_ext own
            l2s, l2b = ln_params(ln2_s[l], ln2_b[l], trivial["ln2_s"],
                                 trivial["ln2_b"])
            f2b = free_bias(bf2[l], trivial["bf2"], blocks=3)
            b1c = bias_cols(bf1[l], trivial["bf1"], nd=NF, tag="bf1_cols")
            wr1, wr2 = _r(Wf1[l]), _r(Wf2[l])
            for hf in range(2):
                t0 = hf * (NT_OWN // 2)
                transpose_to(
                    x2T,
                    lambda tt, dt: x2[:, t0 + tt, dt * 128:(dt + 1) * 128],
                    NT_OWN // 2)
                for m in range(NF):
                    wt = WP.tile([128, ND, 128], BF16, tag="w_lhsT")
                    nc.sync.dma_start(wt[:], wr1[:, :, m * 128:(m + 1) * 128])
                    ps = PSB.tile([128, 512], F32, tag="ps512", name="ps512")
                    for k in range(ND):
                        nc.tensor.matmul(ps[:], wt[:, k, :], x2T[:, k, :],
                                         start=(k == 0), stop=(k == ND - 1))
                    nc.scalar.activation(
                        h1T[:, m, :], ps[:], AF.Gelu,
                        bias=b1c[:, m:m + 1] if b1c is not None else 0.0,
                        scale=1.0)
                for nb in range(3):
                    wt2 = wf2_buf[nb % 2]
                    nc.sync.dma_start(wt2[:, :, :],
                                      wr2[:, :, nb * 256:(nb + 1) * 256])
                    for tt in range(NT_OWN // 2):
                        ps = PSB.tile([128, 512], F32, tag="ps512",
                                      name="ps512")[:, 0:256]
                        for k in range(NF):
                            nc.tensor.matmul(
                                ps, h1T[:, k, tt * 128:(tt + 1) * 128],
                                wt2[:, k, :], start=(k == 0),
                                stop=(k == NF - 1))
                        if f2b is not None:
                            nc.vector.tensor_add(ps, ps, f2b[:, nb, :])
                        nc.vector.tensor_add(
                            x_ext[:, 2 + t0 + tt, nb * 256:(nb + 1) * 256],
                            ps, x2[:, t0 + tt, nb * 256:(nb + 1) * 256])
                for tt in range(NT_OWN // 2):
                    layernorm(x_ext[:, 2 + t0 + tt, :],
                              x_ext[:, 2 + t0 + tt, :], l2s, l2b)

        except _StopEmit:
            pass
        if debug:
            nc.sync.dma_start(
                x_dbg[L].rearrange("(t p) d -> p t d", p=128), x_ext[:])

        # ================= pooling + classifier =================
        pooled = TMP.tile([128, ND], F32, tag="pooled")
        for dt in range(ND):
            ps = PSA.tile([128, 256], F32, tag="ps256", name="ps256")[:, 0:1]
            for tt in range(NT_OWN):
                nc.tensor.matmul(ps, x_ext[:, 2 + tt, dt * 128:(dt + 1) * 128],
                                 amask_sb[:, tt:tt + 1], start=(tt == 0),
                                 stop=(tt == NT_OWN - 1))
            nc.vector.tensor_copy(pooled[:, dt:dt + 1], ps)
        nc.sync.dma_start(pool_in[:], pooled[:])
        nc.gpsimd.collective_compute(
            "AllReduce", ALU.add, replica_groups=groups_b,
            ins=[pool_in.opt()], outs=[pool_out.opt()])
        pooled_r = TMP.tile([128, ND], F32, tag="pooled_r")
        nc.sync.dma_start(pooled_r[:], pool_out[:])
        cw = PS.tile([128, ND, NL], F32)
        nc.sync.dma_start(cw[:], _r(clf_w))
        cb = PS.tile([NL, 1], F32)
        nc.sync.dma_start(cb[:], clf_b[:, None])
        csc = PS.tile([NL, 1], F32)
        nc.sync.dma_start(csc[:], clfscale[:])
        lps = PSA.tile([128, 256], F32, tag="ps256", name="ps256")[0:NL, 0:1]
        for dt in range(ND):
            nc.tensor.matmul(lps, cw[:, dt, :], pooled_r[:, dt:dt + 1],
                             start=(dt == 0), stop=(dt == ND - 1))
        lg = TMP.tile([NL, 1], F32, tag="logits_sb")
        nc.scalar.activation(lg[:], lps, AF.Identity, bias=cb[:], scale=csc[:])
        nc.sync.dma_start(logits_out[:], lg[:])

    nc.compile()
    return nc


# ---------------------------------------------------------------- host side
_CACHE = {}
last_result = None


def _host_prep(inputs):
    ii = {k: np.asarray(v) for k, v in inputs.items()}
    emb = ii["word_emb"][ii["input_ids"]] + ii["pos_emb"][None]
    emb = np.ascontiguousarray(emb, dtype=np.float32)
    am = ii["attention_mask"].astype(np.int32)

    trivial = {
        "bq": not ii["bq"].any(), "bk": not ii["bk"].any(),
        "bv": not ii["bv"].any(), "bqg": not ii["bqg"].any(),
        "bkg": not ii["bkg"].any(), "bvg": not ii["bvg"].any(),
        "bo": not ii["bo"].any(), "bf1": not ii["bf1"].any(),
        "bf2": not ii["bf2"].any(),
        "ln1_s": bool((ii["ln1_s"] == 1).all()), "ln1_b": not ii["ln1_b"].any(),
        "ln2_s": bool((ii["ln2_s"] == 1).all()), "ln2_b": not ii["ln2_b"].any(),
        "emb_ln_s": bool((ii["emb_ln_s"] == 1).all()),
        "emb_ln_b": not ii["emb_ln_b"].any(),
    }

    bf = ml_dtypes.bfloat16
    shared = {}
    for k in ("Wq", "Wk", "Wv", "Wqg", "Wkg", "Wvg", "Wo", "Wf1", "Wf2"):
        shared[k] = np.ascontiguousarray(ii[k].astype(bf))
    for k in ("bk", "bkg", "bv", "bvg", "bo", "bf1", "bf2",
              "ln1_s", "ln1_b", "ln2_s", "ln2_b", "emb_ln_s", "emb_ln_b",
              "clf_w", "clf_b"):
        shared[k] = np.ascontiguousarray(ii[k], dtype=np.float32)
    shared["bq_s"] = (ii["bq"] * 0.125).astype(np.float32)
    shared["bqg_s"] = (ii["bqg"] * 0.125).astype(np.float32)

    bandmask = np.zeros((128, 4, C), np.float32)
    for j, kt in enumerate(BAND_KTS):
        kj = kt * 128 + np.arange(128)[:, None]
        qi = np.arange(C)[None, :]
        ok = (qi <= kj) & (qi >= kj - 2 * C)
        bandmask[:, j, :] = np.where(ok, 0.0, NEG)

    in_maps = []
    for c in range(N_CORES):
        b, q4 = c // 4, c % 4
        t0 = q4 * OWN
        m = dict(shared)
        m["emb_own"] = np.ascontiguousarray(emb[b, t0:t0 + OWN])
        bc = np.zeros((128, NCH_OWN * 6), np.float32)
        for i in range(NCH_OWN):
            n = q4 * NCH_OWN + i
            for kt in range(6):
                kidx = n * C - C + kt * 128 + np.arange(128)
                bad = (kidx < 0) | (kidx >= S) | (kidx < G)
                bad |= np.where(
                    (kidx >= 0) & (kidx < S),
                    am[b].take(np.clip(kidx, 0, S - 1)) == 0, False)
                bc[:, i * 6 + kt] = np.where(bad, NEG, 0.0)
        m["biascols"] = bc
        m["bandmask"] = bandmask
        m["maskG"] = np.where(am[b, :G] != 0, 0.0, NEG).astype(
            np.float32).reshape(G, 1)
        ownm = am[b, t0:t0 + OWN].reshape(NT_OWN, 128).T
        m["mask_og"] = np.ascontiguousarray(
            np.where(ownm != 0, 0.0, NEG).astype(np.float32))
        m["amask"] = np.ascontiguousarray(ownm.astype(np.float32))
        sg = 1.0 if q4 == 0 else 0.0
        m["selg_f"] = np.tile(np.array([[1.0 - sg, sg]], np.float32), (128, 1))
        m["selg_bf"] = m["selg_f"].astype(bf)
        denom = float(am[b].sum())
        m["clfscale"] = np.full((NL, 1), 1.0 / denom, np.float32)
        off = np.zeros((1, 12), np.int32)
        off[0, 0] = (c - 1) * 2 * C + C if q4 > 0 else c * 2 * C
        off[0, 1] = (c + 1) * 2 * C if q4 < 3 else c * 2 * C
        off[0, 2] = (4 * b) * 2 * C
        for s in range(4):
            off[0, 3 + s] = (4 * b + s) * 65
        m["offs"] = off
        in_maps.append(m)
    return in_maps, trivial, ii


def build(trivial=None, debug=None):
    if trivial is None:
        trivial = {k: True for k in (
            "bq", "bk", "bv", "bqg", "bkg", "bvg", "bo", "bf1", "bf2",
            "ln1_s", "ln1_b", "ln2_s", "ln2_b", "emb_ln_s", "emb_ln_b")}
    if debug is None:
        debug = DEBUG
    key = (tuple(sorted(trivial.items())), debug, KLEVEL)
    if key not in _CACHE:
        _CACHE[key] = build_program(trivial, debug=debug)
    return _CACHE[key]


def kernel(**inputs):
    global last_result
    from concourse import bass_utils
    in_maps, trivial, ii = _host_prep(inputs)
    nc = build(trivial)
    res = bass_utils.run_bass_kernel_spmd(
        nc, in_maps, core_ids=list(range(N_CORES)), trace=False)
    last_result = res
    logits = np.stack([res.results[0]["logits_out"][:, 0],
                       res.results[4]["logits_out"][:, 0]])
    z = logits.astype(np.float32)
    y = np.asarray(ii["labels"], np.float32)
    loss = np.mean(np.maximum(z, 0.0) - z * y
                   + np.log1p(np.exp(-np.abs(z)))).astype(np.float32)
    return logits, loss


# revision 27
# speedup vs baseline: 1.0194x; 1.0194x over previous
"""Trainium2 Bass kernel for nn_ClinicalLongformerPool (8 NeuronCores, SPMD).

Sharding: 2-way data parallel over batch x 4-way sequence parallel.
Core c owns tokens [q*1024, (q+1)*1024) of batch b, where b=c//4, q=c%4.
Per layer one 8-core AllGather exchanges 256-token halo edges (+ the 32
global tokens); global-token full-sequence attention is merged flash-style
via a small second AllGather; pooling uses a batch-grouped AllReduce.

Numerics: matmul operands in bf16, accumulation / softmax sums / residual
stream / layernorm in fp32.  Scores are tiny (|s| < ~5) so softmax runs
without a max-subtraction pass; masking adds -30000 before exp (exp -> 0).
"""
import sys
import types

import numpy as np


# ---------------------------------------------------------------- NTFF hook
def _install_ntff_hook():
    try:
        from antenv.axon_hooks import get_axon_ntff_profile_hook  # noqa: F401
        return
    except ImportError:
        pass
    try:
        import antenv
        from trn_agent_boot.trn_boot import _ntff_profile_via_ctypes
    except ImportError:
        return
    mod = types.ModuleType("antenv.axon_hooks")
    _h = [None]
    mod.set_axon_ntff_profile_hook = lambda h: _h.__setitem__(0, h)
    mod.get_axon_ntff_profile_hook = lambda: _h[0]
    sys.modules["antenv.axon_hooks"] = mod
    antenv.axon_hooks = mod
    try:
        mod.set_axon_ntff_profile_hook(
            _ntff_profile_via_ctypes("/opt/axon/libaxon_pjrt.so"))
    except Exception:
        pass


_install_ntff_hook()

import ml_dtypes  # noqa: E402
import concourse.bass as bass  # noqa: E402
import concourse.bacc as bacc  # noqa: E402
import concourse.tile as tile  # noqa: E402
import concourse.mybir as mybir  # noqa: E402
from concourse.bass import ds  # noqa: E402
from concourse.masks import make_identity  # noqa: E402
from concourse.tile_rust import add_dep_helper as _adh  # noqa: E402


def add_dep_helper(a, b, reason=""):
    _adh(getattr(a, "ins", a), getattr(b, "ins", b), reason=reason)

F32 = mybir.dt.float32
BF16 = mybir.dt.bfloat16
I32 = mybir.dt.int32
AF = mybir.ActivationFunctionType
ALU = mybir.AluOpType

# model dims
B, S, D, H, DH, L = 2, 4096, 768, 12, 64, 2
C = 256            # chunk / one-sided window
G = 32             # global tokens
FF = 3072
NL = 25
OWN = S // 4       # 1024 tokens per core
EXT = OWN + 2 * C  # 1536
NT_OWN = OWN // 128   # 8
NT_EXT = EXT // 128   # 12
ND = D // 128         # 6
NF = FF // 128        # 24
NCH_OWN = OWN // C    # 4 chunks per core
NEG = -30000.0
N_CORES = 8

BAND_KTS = (0, 1, 4, 5)   # score k-tiles needing the (static) band mask

DEBUG = False
import os  # noqa: E402
KLEVEL = int(os.environ.get("KLEVEL", "99"))


class _StopEmit(Exception):
    pass


def _stop_if(n):
    if KLEVEL <= n:
        raise _StopEmit()


def _r(w):  # [ (t p), n ] -> [p, t, n] view for K-partition tiling
    return w.rearrange("(t p) n -> p t n", p=128)


def _pbcast(ap, p):
    """Partition-broadcast a [1, ...] AP to [p, ...] (step-0 partition dim)."""
    return bass.AP(tensor=ap.tensor, offset=ap.offset,
                   ap=[[0, p]] + [list(x) for x in ap.ap[1:]])


def _hbcast(ap2):
    """[2, N] AP -> [128, N]: rows 0:64 <- row 0, rows 64:128 <- row 1."""
    s0 = list(ap2.ap[0])[0]
    return bass.AP(tensor=ap2.tensor, offset=ap2.offset,
                   ap=[[s0, 2], [0, 64]] + [list(x) for x in ap2.ap[1:]])


def build_program(trivial, debug=False):
    nc = bacc.Bacc("TRN2", target_bir_lowering=False, debug=False,
                   num_devices=N_CORES)

    def inp(name, shape, dt=F32):
        return nc.dram_tensor(name, list(shape), dt, kind="ExternalInput").ap()

    # replicated weights (host pre-casts the matmul weights to bf16)
    Wq, Wk, Wv = (inp(n, (L, D, D), BF16) for n in ("Wq", "Wk", "Wv"))
    Wqg, Wkg, Wvg = (inp(n, (L, D, D), BF16) for n in ("Wqg", "Wkg", "Wvg"))
    Wo = inp("Wo", (L, D, D), BF16)
    Wf1 = inp("Wf1", (L, D, FF), BF16)
    Wf2 = inp("Wf2", (L, FF, D), BF16)
    bq_s, bqg_s = inp("bq_s", (L, D)), inp("bqg_s", (L, D))  # pre-scaled 1/8
    bk, bkg = inp("bk", (L, D)), inp("bkg", (L, D))
    bv, bvg, bo = inp("bv", (L, D)), inp("bvg", (L, D)), inp("bo", (L, D))
    bf1, bf2 = inp("bf1", (L, FF)), inp("bf2", (L, D))
    ln1_s, ln1_b = inp("ln1_s", (L, D)), inp("ln1_b", (L, D))
    ln2_s, ln2_b = inp("ln2_s", (L, D)), inp("ln2_b", (L, D))
    eln_s, eln_b = inp("emb_ln_s", (D,)), inp("emb_ln_b", (D,))
    clf_w, clf_b = inp("clf_w", (D, NL)), inp("clf_b", (NL,))
    # per-core tensors
    emb_own = inp("emb_own", (OWN, D))
    biascols = inp("biascols", (128, NCH_OWN * 6))   # per (chunk, kt) exp bias
    bandmask = inp("bandmask", (128, 4, C))      # static band masks
    maskG = inp("maskG", (G, 1))                     # global-key mask (additive)
    mask_og = inp("mask_og", (128, NT_OWN))          # og key mask (additive)
    amask = inp("amask", (128, NT_OWN))              # pooling weights (0/1)
    selg_bf = inp("selg_bf", (128, 2), BF16)         # [1-selg, selg]
    selg_f = inp("selg_f", (128, 2))
    clfscale = inp("clfscale", (NL, 1))              # 1/denom
    offs = inp("offs", (1, 12), I32)

    logits_out = nc.dram_tensor("logits_out", [NL, 1], F32,
                                kind="ExternalOutput").ap()
    if debug:
        x_dbg = nc.dram_tensor("x_dbg", [L + 1, EXT, D], F32,
                               kind="ExternalOutput").ap()
        o_dbg = nc.dram_tensor("o_dbg", [L, D, OWN], F32,
                               kind="ExternalOutput").ap()

    groups8 = [list(range(N_CORES))]
    groups_b = [[0, 1, 2, 3], [4, 5, 6, 7]]

    with tile.TileContext(nc) as tc, \
         nc.allow_low_precision(reason="bf16 matmul operands are intentional"), \
         tc.tile_pool(name="persist", bufs=1) as PS, \
         tc.tile_pool(name="acts", bufs=1) as ACTS, \
         tc.tile_pool(name="wpool", bufs=2) as WP, \
         tc.tile_pool(name="tmp", bufs=2) as TMP, \
         tc.tile_pool(name="attp", bufs=4) as ATP, \
         tc.tile_pool(name="ogp", bufs=1) as OGP, \
         tc.tile_pool(name="dram", bufs=1, space="DRAM") as DRAM, \
         tc.tile_pool(name="ps512", bufs=3, space="PSUM") as PSB, \
         tc.tile_pool(name="ps256", bufs=3, space="PSUM") as PSA, \
         tc.tile_pool(name="ps_out", bufs=2, space="PSUM") as PSO:

        # ---------------- persistent small tiles
        ident = PS.tile([128, 128], F32)
        make_identity(nc, ident[:])
        eps_t = PS.tile([128, 1], F32)
        nc.vector.memset(eps_t[:], 1e-5)
        band_sb = PS.tile([128, 4, C], F32)
        nc.sync.dma_start(band_sb[:], bandmask[:])
        bcols_sb = PS.tile([128, NCH_OWN * 6], F32)
        nc.sync.dma_start(bcols_sb[:], biascols[:])
        maskG_sb = PS.tile([G, 1], F32)
        nc.sync.dma_start(maskG_sb[:], maskG[:])
        mog_sb = PS.tile([128, NT_OWN], F32)
        nc.sync.dma_start(mog_sb[:], mask_og[:])
        amask_sb = PS.tile([128, NT_OWN], F32)
        nc.sync.dma_start(amask_sb[:], amask[:])
        selgb_sb = PS.tile([128, 2], BF16)
        nc.sync.dma_start(selgb_sb[:], selg_bf[:])
        selgf_sb = PS.tile([128, 2], F32)
        nc.sync.dma_start(selgf_sb[:], selg_f[:])

        # dynamic offsets
        def load_off(i, maxv):
            regs = nc.alloc_registers(f"off{i}")
            nc.regs_load(regs, offs[0:1, i:i + 1])
            return nc.snap(regs, donate=True, min_val=0, max_val=maxv)

        off_left = load_off(0, N_CORES * 2 * C - C)
        off_right = load_off(1, N_CORES * 2 * C - C)
        off_g32 = load_off(2, N_CORES * 2 * C - G)
        off_fl = [load_off(3 + s, N_CORES * 65 - 65) for s in range(4)]

        # ---------------- big activations
        x_ext = ACTS.tile([128, NT_EXT, D], F32)      # residual stream (+halo)
        xg32 = ACTS.tile([G, D], F32)
        xgT = ACTS.tile([128, ND, G], BF16)
        qgT = ACTS.tile([128, ND, G], BF16)
        kGT = ACTS.tile([128, ND, G], BF16)
        vG_aug = ACTS.tile([G, H, DH + 1], BF16)
        attn_oT = ACTS.tile([128, ND, OWN], BF16)
        sums_sb = ACTS.tile([H, OWN], F32)
        recips = ACTS.tile([H, OWN], BF16)
        og_resh = ACTS.tile([128, ND, G], BF16)

        # aliased groups (disjoint lifetimes share one buffer)
        BIGA = ACTS.tile([128, 12384], BF16)
        kgT = BIGA[:, 0:6144].rearrange("p (k n) -> p k n", k=ND)
        qT = BIGA[:, 0:6144].rearrange("p (k n) -> p k n", k=ND)
        vg_aug = BIGA[:, 6144:12384].rearrange(
            "p (t h d) -> p t h d", t=NT_OWN, h=H)
        recip_bc = BIGA[:, 6144:12288].rearrange("p (k n) -> p k n", k=ND)
        x2 = BIGA[:, 0:12288].bitcast(F32).rearrange(
            "p (t d) -> p t d", t=NT_OWN)          # fp32 view, 8x768

        BIGB = ACTS.tile([128, 15360], BF16)
        xT = BIGB[:, 0:9216].rearrange("p (k n) -> p k n", k=ND)
        attn_stage = BIGB[:, 9216:15360]           # unused scratch
        x2T = BIGB[:, 0:3072].rearrange("p (k n) -> p k n", k=ND)
        h1T = BIGB[:, 3072:15360].rearrange("p (k n) -> p k n", k=NF)

        BIGC = ACTS.tile([128, 18576], BF16)
        kT = BIGC[:, 0:9216].rearrange("p (k n) -> p k n", k=ND)
        v_aug = BIGC[:, 9216:18576].rearrange(
            "p (t h d) -> p t h d", t=NT_EXT, h=H)
        wf2_buf = [BIGC[:, 0:6144].rearrange("p (k n) -> p k n", k=NF),
                   BIGC[:, 6144:12288].rearrange("p (k n) -> p k n", k=NF)]

        # DRAM bounce buffers
        edge_in = DRAM.tile([2 * C, D], F32)
        edge_outs = [DRAM.tile([N_CORES * 2 * C, D], F32, addr_space="Shared",
                               name=f"edge_out{i}") for i in range(L)]
        fl_in = DRAM.tile([65, H * G], F32)
        fl_outs = [DRAM.tile([N_CORES * 65, H * G], F32, addr_space="Shared",
                             name=f"fl_out{i}") for i in range(L)]
        pool_in = DRAM.tile([128, ND], F32)
        pool_out = DRAM.tile([128, ND], F32)
        ogrec_dr = DRAM.tile([1, H * G], F32)
        recips_dr = DRAM.tile([H, OWN], BF16)
        sums_dr = DRAM.tile([H, OWN], F32)

        # ---------------- helpers
        def layernorm(src_ap, dst_ap, s_bc, b_bc):
            stats = TMP.tile([128, 3, 6], F32, tag="ln_stats")
            mv = TMP.tile([128, 2], F32, tag="ln_mv")
            for sg in range(3):
                nc.vector.bn_stats(stats[:, sg, :],
                                   src_ap[:, sg * 256:(sg + 1) * 256])
            nc.vector.bn_aggr(mv[:], stats[:])
            nc.scalar.activation(mv[:, 1:2], mv[:, 1:2], AF.Sqrt,
                                 bias=eps_t[:], scale=1.0)
            nc.vector.reciprocal(mv[:, 1:2], mv[:, 1:2])
            nc.vector.tensor_scalar(dst_ap, src_ap, mv[:, 0:1], mv[:, 1:2],
                                    ALU.subtract, ALU.mult)
            if s_bc is not None:
                nc.vector.tensor_mul(dst_ap, dst_ap, s_bc[:])
            if b_bc is not None:
                nc.vector.tensor_add(dst_ap, dst_ap, b_bc[:])

        def ln_params(s_in, b_in, s_triv, b_triv):
            s_bc = b_bc = None
            if not s_triv:
                s_bc = TMP.tile([128, D], F32, tag="ln_sbc")
                nc.sync.dma_start(s_bc[:], _pbcast(s_in[None, :], 128))
            if not b_triv:
                b_bc = TMP.tile([128, D], F32, tag="ln_bbc")
                nc.sync.dma_start(b_bc[:], _pbcast(b_in[None, :], 128))
            return s_bc, b_bc

        def bias_cols(b_in, triv, nd=ND, tag="bias_cols"):
            if triv:
                return None
            t = TMP.tile([128, nd], F32, tag=tag)
            nc.sync.dma_start(t[:], b_in.rearrange("(t p) -> p t", p=128))
            return t

        def free_bias(b_in, triv, blocks=2):
            if triv:
                return None
            t = TMP.tile([128, blocks, D // blocks], F32,
                         tag=f"free_bias{blocks}")
            nc.sync.dma_start(
                t[:], _pbcast(b_in.rearrange("(a n) -> a n", a=blocks)[None],
                              128))
            return t

        def proj_T(out_t, w, b_col, rhs_t, ntok, scale=1.0, rhs_off=0):
            """out_t[:, m, :ntok] = w[ktile].T @ rhs (+bias, *scale)."""
            wr = _r(w)
            for m in range(ND):
                wt = WP.tile([128, ND, 128], BF16, tag="w_lhsT")
                nc.sync.dma_start(wt[:], wr[:, :, m * 128:(m + 1) * 128])
                for nb0 in range(0, ntok, 512):
                    nn = min(512, ntok - nb0)
                    ps = PSB.tile([128, 512], F32, tag="ps512", name="ps512")
                    for k in range(ND):
                        nc.tensor.matmul(
                            ps[:, :nn], wt[:, k, :],
                            rhs_t[:, k, rhs_off + nb0:rhs_off + nb0 + nn],
                            start=(k == 0), stop=(k == ND - 1))
                    nc.scalar.activation(
                        out_t[:, m, nb0:nb0 + nn], ps[:, :nn], AF.Identity,
                        bias=b_col[:, m:m + 1] if b_col is not None else 0.0,
                        scale=scale)

        def proj_V(out_t, w, lhs_t, ntt, badd, lhs_off=0):
            """v-style: out_t[:, tt, h, 0:64] = x @ w (+b); ones in col 64."""
            wr = _r(w)
            for nb in range(2):
                wt = WP.tile([128, ND, 384], BF16, tag="w_rhs")
                nc.sync.dma_start(wt[:], wr[:, :, nb * 384:(nb + 1) * 384])
                for tt in range(ntt):
                    c0 = lhs_off + tt * 128
                    ps = PSB.tile([128, 512], F32, tag="ps512", name="ps512")[:, 0:384]
                    for k in range(ND):
                        nc.tensor.matmul(
                            ps[:], lhs_t[:, k, c0:c0 + 128],
                            wt[:, k, :], start=(k == 0), stop=(k == ND - 1))
                    if badd is not None:
                        nc.vector.tensor_add(ps[:], ps[:], badd[:, nb, :])
                    nc.vector.tensor_copy(
                        out_t[:, tt, nb * 6:(nb + 1) * 6, 0:DH],
                        ps[:].rearrange("p (h d) -> p h d", d=DH))
            nc.vector.memset(out_t[:, :, :, DH:DH + 1], 1.0)

        def transpose_to(dst, src_fn, ntt, dst_off=0):
            for tt in range(ntt):
                for dt in range(ND):
                    pt = PSA.tile([128, 256], F32, tag="ps256", name="ps256")[:, 0:128]
                    nc.tensor.transpose(pt, src_fn(tt, dt), ident[:])
                    nc.scalar.copy(
                        dst[:, dt, dst_off + tt * 128:dst_off + (tt + 1) * 128],
                        pt)

        # ================= embedding layernorm =================
        es_bc, eb_bc = ln_params(eln_s, eln_b, trivial["emb_ln_s"],
                                 trivial["emb_ln_b"])
        er = emb_own.rearrange("(t p) d -> p t d", p=128)
        for tt in (0, 1, 6, 7, 2, 3, 4, 5):
            et = TMP.tile([128, D], F32, tag="emb_in")
            nc.sync.dma_start(et[:], er[:, tt, :])
            layernorm(et[:], x_ext[:, 2 + tt, :], es_bc, eb_bc)

        # ================= layers =================
        try:
         for l in range(L):
            if l == 1:
                _stop_if(98)
            edge_out, fl_out = edge_outs[l], fl_outs[l]
            # ---- edge exchange (own first/last 256 post-LN rows)
            edge_in_v = edge_in.rearrange("(t p) d -> p t d", p=128)
            nc.sync.dma_start(edge_in_v[:, 0:2, :], x_ext[:, 2:4, :])
            nc.sync.dma_start(edge_in_v[:, 2:4, :], x_ext[:, 8:10, :])
            cc_edge = nc.gpsimd.collective_compute(
                "AllGather", ALU.bypass, replica_groups=groups8,
                ins=[edge_in.opt()], outs=[edge_out.opt()])
            r1 = nc.sync.dma_start(
                x_ext[:, 0:2, :],
                edge_out[ds(off_left, C), :].rearrange("(t p) d -> p t d", p=128))
            r2 = nc.sync.dma_start(
                x_ext[:, 10:12, :],
                edge_out[ds(off_right, C), :].rearrange("(t p) d -> p t d", p=128))
            r3 = nc.sync.dma_start(xg32[:], edge_out[ds(off_g32, G), :])
            for r in (r1, r2, r3):
                add_dep_helper(r, cc_edge, reason="dyn read after AG")

            _stop_if(1)
            if debug:
                nc.sync.dma_start(
                    x_dbg[l].rearrange("(t p) d -> p t d", p=128), x_ext[:])

            # ---- transposes
            transpose_to(xT, lambda tt, dt: x_ext[:, tt, dt * 128:(dt + 1) * 128],
                         NT_EXT)
            for dt in range(ND):
                pt = PSA.tile([128, 256], F32, tag="ps256", name="ps256")[:, 0:G]
                nc.tensor.matmul(pt, xg32[:, dt * 128:(dt + 1) * 128],
                                 ident[0:G, 0:G], is_transpose=True,
                                 start=True, stop=True)
                nc.scalar.copy(xgT[:, dt, :], pt)

            _stop_if(2)
            # ---- og projections first (kgT/vg_aug alias qT/recip_bc)
            proj_T(kgT, Wkg[l], bias_cols(bkg[l], trivial["bkg"]),
                   xT, OWN, rhs_off=C)
            proj_T(qgT, Wqg[l], bias_cols(bqg_s[l], trivial["bqg"]),
                   xgT, G, scale=0.125)
            proj_V(vg_aug, Wvg[l], xT, NT_OWN,
                   free_bias(bvg[l], trivial["bvg"]), lhs_off=C)

            _stop_if(3)
            # ---- og flash partials + AllGather + merge
            og_ps = PSO.tile([65, H, G], F32, tag="ps_out", name="ps_out")
            for h in range(H):
                p0, dt = (h % 2) * 64, h // 2
                for kt in range(NT_OWN):
                    ps_s = PSA.tile([128, 256], F32, tag="ps256", name="ps256")[:, 0:G]
                    nc.tensor.matmul(
                        ps_s, kgT[p0:p0 + 64, dt, kt * 128:(kt + 1) * 128],
                        qgT[p0:p0 + 64, dt, :], start=True, stop=True)
                    pg = ATP.tile([128, G], BF16, tag="og_p")
                    nc.scalar.activation(pg[:], ps_s, AF.Exp,
                                         bias=mog_sb[:, kt:kt + 1], scale=1.0)
                    nc.tensor.matmul(og_ps[:, h, :], vg_aug[:, kt, h, :],
                                     pg[:], start=(kt == 0),
                                     stop=(kt == NT_OWN - 1))
            og_part = OGP.tile([65, H * G], F32, tag="og_part")
            nc.vector.tensor_copy(og_part[:],
                                  og_ps[:].rearrange("p h g -> p (h g)"))
            nc.sync.dma_start(fl_in[:], og_part[:])
            cc_fl = nc.gpsimd.collective_compute(
                "AllGather", ALU.bypass, replica_groups=groups8,
                ins=[fl_in.opt()], outs=[fl_out.opt()])
            og_mrg = OGP.tile([65, H * G], F32, tag="og_mrg")
            for s in range(4):
                og_gath = ATP.tile([65, H * G], F32, tag="og_gath",
                                   name="og_gath")
                rg = nc.sync.dma_start(og_gath[:], fl_out[ds(off_fl[s], 65), :])
                add_dep_helper(rg, cc_fl, reason="dyn read after flash AG")
                if s == 0:
                    nc.vector.tensor_copy(og_mrg[:], og_gath[:])
                else:
                    nc.vector.tensor_add(og_mrg[:], og_mrg[:], og_gath[:])
            og_rec = OGP.tile([1, H * G], F32, tag="og_rec")
            nc.vector.reciprocal(og_rec[:], og_mrg[64:65, :])
            w_ogr = nc.sync.dma_start(ogrec_dr[:], og_rec[:])
            og_rbc = OGP.tile([64, H * G], F32, tag="og_rbc")
            r_ogr = nc.sync.dma_start(og_rbc[:], _pbcast(ogrec_dr[0:1, :], 64))
            add_dep_helper(r_ogr, w_ogr, reason="ogrec bounce")
            og_nrm = OGP.tile([64, H * G], F32, tag="og_nrm")
            nc.vector.tensor_mul(og_nrm[:], og_mrg[0:64, :], og_rbc[:])
            nc.vector.tensor_scalar_mul(og_nrm[:], og_nrm[:],
                                        selgf_sb[0:64, 1:2])
            og_fin = OGP.tile([64, H, G], BF16, tag="og_fin")
            nc.vector.tensor_copy(
                og_fin[:].rearrange("p h g -> p (h g)"), og_nrm[:])
            for h in range(H):
                p0, dt = (h % 2) * 64, h // 2
                nc.sync.dma_start(og_resh[p0:p0 + 64, dt, :], og_fin[:, h, :])

            _stop_if(4)
            # ---- main projections
            proj_T(qT, Wq[l], bias_cols(bq_s[l], trivial["bq"]),
                   xT, OWN, scale=0.125, rhs_off=C)
            proj_T(kT, Wk[l], bias_cols(bk[l], trivial["bk"]), xT, EXT)
            proj_T(kGT, Wk[l], bias_cols(bk[l], trivial["bk"]), xgT, G)
            proj_V(v_aug, Wv[l], xT, NT_EXT, free_bias(bv[l], trivial["bv"]))
            # global v (standard Wv of the 32 global tokens)
            wrv = _r(Wv[l])
            vbias = free_bias(bv[l], trivial["bv"])
            for nb in range(2):
                wt = WP.tile([128, ND, 384], BF16, tag="w_rhs")
                nc.sync.dma_start(wt[:], wrv[:, :, nb * 384:(nb + 1) * 384])
                ps = PSB.tile([128, 512], F32, tag="ps512", name="ps512")[0:G, 0:384]
                for k in range(ND):
                    nc.tensor.matmul(ps, xgT[:, k, :], wt[:, k, :],
                                     start=(k == 0), stop=(k == ND - 1))
                if vbias is not None:
                    nc.vector.tensor_add(ps, ps, vbias[0:G, nb, :])
                nc.vector.tensor_copy(
                    vG_aug[:, nb * 6:(nb + 1) * 6, 0:DH],
                    ps.rearrange("p (h d) -> p h d", d=DH))
            nc.vector.memset(vG_aug[:, :, DH:DH + 1], 1.0)

            _stop_if(5)
            # ---- banded local + global-key attention
            for i in range(NCH_OWN):
                for h in range(H):
                    p0, dt = (h % 2) * 64, h // 2
                    qh = qT[p0:p0 + 64, dt, i * C:(i + 1) * C]
                    ps_o = PSO.tile([65, H, G], F32, tag="ps_out", name="ps_out")
                    ps_o = ps_o[:].rearrange("p h g -> p (h g)")[:, 0:C]
                    pTs = []
                    # phase 1: all scores + exps (PE runs ahead of ACT)
                    for kt in range(6):
                        et = i * 2 + kt
                        ps_s = PSA.tile([128, 256], F32, tag="ps256", name="ps256")
                        nc.tensor.matmul(
                            ps_s[:], kT[p0:p0 + 64, dt, et * 128:(et + 1) * 128],
                            qh, start=True, stop=True)
                        if kt in BAND_KTS:
                            j = BAND_KTS.index(kt)
                            nc.vector.tensor_tensor(
                                ps_s[:], ps_s[:], band_sb[:, j, :], ALU.add)
                        pT = ATP.tile([128, C], BF16, tag="att_p", name="att_p", bufs=8)
                        nc.scalar.activation(
                            pT[:], ps_s[:], AF.Exp,
                            bias=bcols_sb[:, i * 6 + kt:i * 6 + kt + 1],
                            scale=1.0)
                        pTs.append(pT)
                    ps_g = PSA.tile([128, 256], F32, tag="ps256", name="ps256")[0:G, :]
                    nc.tensor.matmul(ps_g, kGT[p0:p0 + 64, dt, :], qh,
                                     start=True, stop=True)
                    pgT = ATP.tile([G, C], BF16, tag="att_gp")
                    nc.scalar.activation(pgT[:], ps_g, AF.Exp,
                                         bias=maskG_sb[:], scale=1.0)
                    # phase 2: all AVs
                    for kt in range(6):
                        et = i * 2 + kt
                        nc.tensor.matmul(ps_o, v_aug[:, et, h, :], pTs[kt][:],
                                         start=(kt == 0), stop=False)
                    nc.tensor.matmul(ps_o, vG_aug[:, h, :], pgT[:],
                                     start=False, stop=True)
                    nc.scalar.copy(attn_oT[p0:p0 + 64, dt, i * C:(i + 1) * C],
                                   ps_o[0:64, :])
                    sums_tmp = ATP.tile([1, C], F32, tag="sums_tmp",
                                        name="sums_tmp")
                    nc.vector.tensor_copy(sums_tmp[:], ps_o[64:65, :])
                    nc.sync.dma_start(sums_dr[h:h + 1, i * C:(i + 1) * C],
                                      sums_tmp[:])

            # ---- normalize + og blend
            nc.sync.dma_start(sums_sb[:], sums_dr[:])
            nc.vector.reciprocal(recips[:], sums_sb[:])
            w_rec = nc.sync.dma_start(recips_dr[:], recips[:])
            for dt in range(ND):
                r_rec = nc.sync.dma_start(
                    recip_bc[:, dt, :],
                    _hbcast(recips_dr[2 * dt:2 * dt + 2, :]))
                add_dep_helper(r_rec, w_rec, reason="recips bounce")
            nc.vector.tensor_mul(attn_oT[:], attn_oT[:], recip_bc[:])
            nc.vector.tensor_scalar_mul(attn_oT[:, :, 0:G],
                                        attn_oT[:, :, 0:G], selgf_sb[:, 0:1])
            nc.vector.tensor_add(attn_oT[:, :, 0:G], attn_oT[:, :, 0:G],
                                 og_resh[:])

            if debug:
                nc.gpsimd.dma_start(
                    o_dbg[l].rearrange("(k p) n -> p k n", p=128), attn_oT[:])

            _stop_if(6)
            # ---- out-projection + residual + LN1 -> x2 (fp32, aliases BIGA)
            l1s, l1b = ln_params(ln1_s[l], ln1_b[l], trivial["ln1_s"],
                                 trivial["ln1_b"])
            ob = free_bias(bo[l], trivial["bo"])
            wro = _r(Wo[l])
            for nb in range(2):
                wt = WP.tile([128, ND, 384], BF16, tag="w_rhs")
                nc.sync.dma_start(wt[:], wro[:, :, nb * 384:(nb + 1) * 384])
                for tt in range(NT_OWN):
                    ps = PSB.tile([128, 512], F32, tag="ps512", name="ps512")[:, 0:384]
                    for k in range(ND):
                        nc.tensor.matmul(
                            ps, attn_oT[:, k, tt * 128:(tt + 1) * 128],
                            wt[:, k, :], start=(k == 0), stop=(k == ND - 1))
                    if ob is not None:
                        nc.vector.tensor_add(ps, ps, ob[:, nb, :])
                    nc.vector.tensor_add(
                        x2[:, tt, nb * 384:(nb + 1) * 384], ps,
                        x_ext[:, 2 + tt, nb * 384:(nb + 1) * 384])
            for tt in range(NT_OWN):
                layernorm(x2[:, tt, :], x2[:, tt, :], l1s, l1b)

            _stop_if(7)
            # ---- FFN (two token halves) + residual + LN2 -> x_ext own
            l2s, l2b = ln_params(ln2_s[l], ln2_b[l], trivial["ln2_s"],
                                 trivial["ln2_b"])
            f2b = free_bias(bf2[l], trivial["bf2"], blocks=3)
            b1c = bias_cols(bf1[l], trivial["bf1"], nd=NF, tag="bf1_cols")
            wr1, wr2 = _r(Wf1[l]), _r(Wf2[l])
            half_tiles = ((0, 1, 6, 7), (2, 3, 4, 5))
            for hf in range(2):
                tls = half_tiles[hf]
                transpose_to(
                    x2T,
                    lambda tt, dt: x2[:, tls[tt], dt * 128:(dt + 1) * 128],
                    NT_OWN // 2)
                for m in range(NF):
                    wt = WP.tile([128, ND, 128], BF16, tag="w_lhsT")
                    nc.sync.dma_start(wt[:], wr1[:, :, m * 128:(m + 1) * 128])
                    ps = PSB.tile([128, 512], F32, tag="ps512", name="ps512")
                    for k in range(ND):
                        nc.tensor.matmul(ps[:], wt[:, k, :], x2T[:, k, :],
                                         start=(k == 0), stop=(k == ND - 1))
                    nc.scalar.activation(
                        h1T[:, m, :], ps[:], AF.Gelu,
                        bias=b1c[:, m:m + 1] if b1c is not None else 0.0,
                        scale=1.0)
                for nb in range(3):
                    wt2 = wf2_buf[nb % 2]
                    nc.sync.dma_start(wt2[:, :, :],
                                      wr2[:, :, nb * 256:(nb + 1) * 256])
                    for tt in range(NT_OWN // 2):
                        ps = PSB.tile([128, 512], F32, tag="ps512",
                                      name="ps512")[:, 0:256]
                        for k in range(NF):
                            nc.tensor.matmul(
                                ps, h1T[:, k, tt * 128:(tt + 1) * 128],
                                wt2[:, k, :], start=(k == 0),
                                stop=(k == NF - 1))
                        if f2b is not None:
                            nc.vector.tensor_add(ps, ps, f2b[:, nb, :])
                        nc.vector.tensor_add(
                            x_ext[:, 2 + tls[tt], nb * 256:(nb + 1) * 256],
                            ps, x2[:, tls[tt], nb * 256:(nb + 1) * 256])
                for tt in tls:
                    layernorm(x_ext[:, 2 + tt, :],
                              x_ext[:, 2 + tt, :], l2s, l2b)

        except _StopEmit:
            pass
        if debug:
            nc.sync.dma_start(
                x_dbg[L].rearrange("(t p) d -> p t d", p=128), x_ext[:])

        # ================= pooling + classifier =================
        pooled = TMP.tile([128, ND], F32, tag="pooled")
        for dt in range(ND):
            ps = PSA.tile([128, 256], F32, tag="ps256", name="ps256")[:, 0:1]
            for tt in range(NT_OWN):
                nc.tensor.matmul(ps, x_ext[:, 2 + tt, dt * 128:(dt + 1) * 128],
                                 amask_sb[:, tt:tt + 1], start=(tt == 0),
                                 stop=(tt == NT_OWN - 1))
            nc.vector.tensor_copy(pooled[:, dt:dt + 1], ps)
        nc.sync.dma_start(pool_in[:], pooled[:])
        nc.gpsimd.collective_compute(
            "AllReduce", ALU.add, replica_groups=groups_b,
            ins=[pool_in.opt()], outs=[pool_out.opt()])
        pooled_r = TMP.tile([128, ND], F32, tag="pooled_r")
        nc.sync.dma_start(pooled_r[:], pool_out[:])
        cw = PS.tile([128, ND, NL], F32)
        nc.sync.dma_start(cw[:], _r(clf_w))
        cb = PS.tile([NL, 1], F32)
        nc.sync.dma_start(cb[:], clf_b[:, None])
        csc = PS.tile([NL, 1], F32)
        nc.sync.dma_start(csc[:], clfscale[:])
        lps = PSA.tile([128, 256], F32, tag="ps256", name="ps256")[0:NL, 0:1]
        for dt in range(ND):
            nc.tensor.matmul(lps, cw[:, dt, :], pooled_r[:, dt:dt + 1],
                             start=(dt == 0), stop=(dt == ND - 1))
        lg = TMP.tile([NL, 1], F32, tag="logits_sb")
        nc.scalar.activation(lg[:], lps, AF.Identity, bias=cb[:], scale=csc[:])
        nc.sync.dma_start(logits_out[:], lg[:])

    nc.compile()
    return nc


# ---------------------------------------------------------------- host side
_CACHE = {}
last_result = None


def _host_prep(inputs):
    ii = {k: np.asarray(v) for k, v in inputs.items()}
    emb = ii["word_emb"][ii["input_ids"]] + ii["pos_emb"][None]
    emb = np.ascontiguousarray(emb, dtype=np.float32)
    am = ii["attention_mask"].astype(np.int32)

    trivial = {
        "bq": not ii["bq"].any(), "bk": not ii["bk"].any(),
        "bv": not ii["bv"].any(), "bqg": not ii["bqg"].any(),
        "bkg": not ii["bkg"].any(), "bvg": not ii["bvg"].any(),
        "bo": not ii["bo"].any(), "bf1": not ii["bf1"].any(),
        "bf2": not ii["bf2"].any(),
        "ln1_s": bool((ii["ln1_s"] == 1).all()), "ln1_b": not ii["ln1_b"].any(),
        "ln2_s": bool((ii["ln2_s"] == 1).all()), "ln2_b": not ii["ln2_b"].any(),
        "emb_ln_s": bool((ii["emb_ln_s"] == 1).all()),
        "emb_ln_b": not ii["emb_ln_b"].any(),
    }

    bf = ml_dtypes.bfloat16
    shared = {}
    for k in ("Wq", "Wk", "Wv", "Wqg", "Wkg", "Wvg", "Wo", "Wf1", "Wf2"):
        shared[k] = np.ascontiguousarray(ii[k].astype(bf))
    for k in ("bk", "bkg", "bv", "bvg", "bo", "bf1", "bf2",
              "ln1_s", "ln1_b", "ln2_s", "ln2_b", "emb_ln_s", "emb_ln_b",
              "clf_w", "clf_b"):
        shared[k] = np.ascontiguousarray(ii[k], dtype=np.float32)
    shared["bq_s"] = (ii["bq"] * 0.125).astype(np.float32)
    shared["bqg_s"] = (ii["bqg"] * 0.125).astype(np.float32)

    bandmask = np.zeros((128, 4, C), np.float32)
    for j, kt in enumerate(BAND_KTS):
        kj = kt * 128 + np.arange(128)[:, None]
        qi = np.arange(C)[None, :]
        ok = (qi <= kj) & (qi >= kj - 2 * C)
        bandmask[:, j, :] = np.where(ok, 0.0, NEG)

    in_maps = []
    for c in range(N_CORES):
        b, q4 = c // 4, c % 4
        t0 = q4 * OWN
        m = dict(shared)
        m["emb_own"] = np.ascontiguousarray(emb[b, t0:t0 + OWN])
        bc = np.zeros((128, NCH_OWN * 6), np.float32)
        for i in range(NCH_OWN):
            n = q4 * NCH_OWN + i
            for kt in range(6):
                kidx = n * C - C + kt * 128 + np.arange(128)
                bad = (kidx < 0) | (kidx >= S) | (kidx < G)
                bad |= np.where(
                    (kidx >= 0) & (kidx < S),
                    am[b].take(np.clip(kidx, 0, S - 1)) == 0, False)
                bc[:, i * 6 + kt] = np.where(bad, NEG, 0.0)
        m["biascols"] = bc
        m["bandmask"] = bandmask
        m["maskG"] = np.where(am[b, :G] != 0, 0.0, NEG).astype(
            np.float32).reshape(G, 1)
        ownm = am[b, t0:t0 + OWN].reshape(NT_OWN, 128).T
        m["mask_og"] = np.ascontiguousarray(
            np.where(ownm != 0, 0.0, NEG).astype(np.float32))
        m["amask"] = np.ascontiguousarray(ownm.astype(np.float32))
        sg = 1.0 if q4 == 0 else 0.0
        m["selg_f"] = np.tile(np.array([[1.0 - sg, sg]], np.float32), (128, 1))
        m["selg_bf"] = m["selg_f"].astype(bf)
        denom = float(am[b].sum())
        m["clfscale"] = np.full((NL, 1), 1.0 / denom, np.float32)
        off = np.zeros((1, 12), np.int32)
        off[0, 0] = (c - 1) * 2 * C + C if q4 > 0 else c * 2 * C
        off[0, 1] = (c + 1) * 2 * C if q4 < 3 else c * 2 * C
        off[0, 2] = (4 * b) * 2 * C
        for s in range(4):
            off[0, 3 + s] = (4 * b + s) * 65
        m["offs"] = off
        in_maps.append(m)
    return in_maps, trivial, ii


def build(trivial=None, debug=None):
    if trivial is None:
        trivial = {k: True for k in (
            "bq", "bk", "bv", "bqg", "bkg", "bvg", "bo", "bf1", "bf2",
            "ln1_s", "ln1_b", "ln2_s", "ln2_b", "emb_ln_s", "emb_ln_b")}
    if debug is None:
        debug = DEBUG
    key = (tuple(sorted(trivial.items())), debug, KLEVEL)
    if key not in _CACHE:
        _CACHE[key] = build_program(trivial, debug=debug)
    return _CACHE[key]


def kernel(**inputs):
    global last_result
    from concourse import bass_utils
    in_maps, trivial, ii = _host_prep(inputs)
    nc = build(trivial)
    res = bass_utils.run_bass_kernel_spmd(
        nc, in_maps, core_ids=list(range(N_CORES)), trace=False)
    last_result = res
    logits = np.stack([res.results[0]["logits_out"][:, 0],
                       res.results[4]["logits_out"][:, 0]])
    z = logits.astype(np.float32)
    y = np.asarray(ii["labels"], np.float32)
    loss = np.mean(np.maximum(z, 0.0) - z * y
                   + np.log1p(np.exp(-np.abs(z)))).astype(np.float32)
    return logits, loss
